# revision 1
# baseline (speedup 1.0000x reference)
"""AutoCompleteDecoderModel (LSTM enc-dec + CE loss) on 8 Trainium2 cores.

v2 strategy (B=256, S=512, H=512, V=128; 8 cores x 32 samples):
 - gates.T [2048, 32] per step in PSUM; gate m-tile order [g|i|f|o]; bank A
   holds m 0..7 (g,i), bank B m 8..15 (f,o).
 - Flights of 2 steps share one PSUM bank pair: the input projection
   (one-hot, bias folded) for both steps of a flight is ONE matmul per
   m-tile (N=64), cutting inproj matmul count 2x.
 - Recurrent matmuls use fp8e4 weights in DoubleRow mode: K=256 per
   instruction -> 32 matmuls/step instead of 64.  h state (hT=2h) is
   stored fp8e4; weights are scaled x64 (and i/f/o rows by an extra 0.5 so
   all four gates use a single tanh scale), unscaled in the ACT pre-scale.
 - Samples split into two anti-phase groups of 16: each group's activation
   chain (1 tanh [128,256], 3 STT, tanh(c), STT) overlaps the other group's
   matmuls, hiding the chain latency behind the recurrence of the peer.
 - Decoder tail batched over groups of 4 steps: logits.T via 4 matmuls
   (N=128) reading an hT ring, one exp [128,128], one tgt-dot mul, one
   ones-matmul reduce [1,256], one copy into the result accumulator.
 - Host: nll = ln(sumexp) - (tgtdot + proj_b[tgt]), masked mean, sum.
"""

import os
import sys

import numpy as np

if "/opt/trn_rl_repo" not in sys.path:
    sys.path.insert(0, "/opt/trn_rl_repo")

B, S, H, V = 256, 512, 512, 128
NCORES = 8
BS = B // NCORES   # 32 samples per core
U = 128            # steps per hw-loop iteration
FS = 2             # steps per PSUM flight
DG = 4             # steps per decoder tail group
WS = 64.0          # fp8 weight scale

_CACHE = {}

_PERM = None


def _perm():
    global _PERM
    if _PERM is None:
        _PERM = np.concatenate([
            np.arange(1024, 1536),  # g
            np.arange(0, 512),      # i
            np.arange(512, 1024),   # f
            np.arange(1536, 2048),  # o
        ])
    return _PERM


def _prep_weights(W_ih, W_hh, b_ih, b_hh):
    """Fold biases into W_ih, fold the hT=2h and single-tanh-scale factors,
    scale x64, quantize fp8e4, and pack for the kernel layouts."""
    import ml_dtypes

    fp8 = ml_dtypes.float8_e4m3
    perm = _perm()
    Wi = (np.asarray(W_ih, np.float64) + (np.asarray(b_ih, np.float64)
          + np.asarray(b_hh, np.float64))[:, None])[perm]  # [2048, 128]
    Wh = 0.5 * np.asarray(W_hh, np.float64)[perm]          # [2048, 512]
    Wi[512:] *= 0.5  # i,f,o rows: single tanh scale (tanh(z/2))
    Wh[512:] *= 0.5
    Wi *= WS
    Wh *= WS
    # input proj lhsT tiles: wih_t [V=128, 2048], m-tile m at cols m*128
    wih_t = np.ascontiguousarray(Wi.T).astype(fp8)
    # recurrent DoubleRow pairs: whh_dr [128, 8192], block (pk, m) at col
    # (pk*16+m)*256, within block [kk=2, c=128]; k-tile = 2*pk+kk
    Wt = np.ascontiguousarray(Wh.T)                        # [512, 2048]
    whh_dr = (Wt.reshape(2, 2, 128, 16, 128)               # [pk,kk,p,m,c]
              .transpose(2, 0, 3, 1, 4)                    # [p,pk,m,kk,c]
              .reshape(128, 8192).astype(fp8))
    return np.ascontiguousarray(wih_t), np.ascontiguousarray(whh_dr)


def _onehot_stream(idx):
    """idx [BS, S] int -> [128, S*32] fp8e4, col t*32+j = (idx[j,t]==v)."""
    import ml_dtypes
    oh = (np.arange(V, dtype=np.int32)[:, None, None]
          == np.asarray(idx, np.int32).T[None, :, :])  # [V, S, BS]
    return np.ascontiguousarray(
        oh.reshape(V, -1).astype(ml_dtypes.float8_e4m3))


def _build_module(n_steps, unrolled=False):
    _ABL_TAIL = bool(int(os.environ.get("ABL_TAIL", "0")))
    import concourse.bacc as bacc
    import concourse.bass as bass
    import concourse.mybir as mybir
    import concourse.tile as tile

    f32 = mybir.dt.float32
    bf16 = mybir.dt.bfloat16
    fp8 = mybir.dt.float8e4
    AF = mybir.ActivationFunctionType
    OP = mybir.AluOpType
    PE = mybir.EngineType.PE
    DR = mybir.MatmulPerfMode.DoubleRow

    assert n_steps % U == 0
    n_iters = n_steps // U

    nc = bacc.Bacc("TRN2", target_bir_lowering=False, debug=False,
                   num_devices=NCORES)

    d_enc_whh = nc.dram_tensor("enc_whh", [128, 8192], fp8, kind="ExternalInput").ap()
    d_enc_wih = nc.dram_tensor("enc_wih", [128, 2048], fp8, kind="ExternalInput").ap()
    d_dec_whh = nc.dram_tensor("dec_whh", [128, 8192], fp8, kind="ExternalInput").ap()
    d_dec_wih = nc.dram_tensor("dec_wih", [128, 2048], fp8, kind="ExternalInput").ap()
    d_projt = nc.dram_tensor("projt", [128, 512], bf16, kind="ExternalInput").ap()
    d_projb = nc.dram_tensor("projb", [128, 1], f32, kind="ExternalInput").ap()
    d_enc_oh = nc.dram_tensor("enc_oh", [128, n_steps * BS], fp8, kind="ExternalInput").ap()
    d_dec_oh = nc.dram_tensor("dec_oh", [128, n_steps * BS], fp8, kind="ExternalInput").ap()
    d_res = nc.dram_tensor("res", [1, (n_steps // DG) * 256], f32, kind="ExternalOutput").ap()

    with tile.TileContext(nc) as tc:
        with (
            tc.tile_pool(name="const", bufs=1) as const_pool,
            tc.tile_pool(name="oh", bufs=3) as oh_pool,
            tc.tile_pool(name="pgA", bufs=3, space="PSUM") as pgA_pool,
            tc.tile_pool(name="pgB", bufs=3, space="PSUM") as pgB_pool,
            tc.tile_pool(name="plog", bufs=1, space="PSUM") as plog_pool,
            tc.tile_pool(name="prsp", bufs=1, space="PSUM") as prsp_pool,
            tc.tile_pool(name="work", bufs=4) as work_pool,
            tc.tile_pool(name="stack", bufs=2) as stack_pool,
            tc.tile_pool(name="acc", bufs=2) as acc_pool,
        ):
            w_enc_hh = const_pool.tile([128, 8192], fp8, tag="wehh")
            w_enc_ih = const_pool.tile([128, 2048], fp8, tag="weih")
            w_dec_hh = const_pool.tile([128, 8192], fp8, tag="wdhh")
            w_dec_ih = const_pool.tile([128, 2048], fp8, tag="wdih")
            w_projt = const_pool.tile([128, 512], bf16, tag="wpt")
            w_projb = const_pool.tile([128, 1], f32, tag="wpb")
            ones_col = const_pool.tile([128, 1], bf16, tag="ones")
            sstA = const_pool.tile([128, 64], f32, tag="sstA")
            sstB = const_pool.tile([128, 64], f32, tag="sstB")

            # Encoder weights first (the first steps need wih then whh);
            # decoder weights + projection load after the encoder loop is
            # issued, overlapping the 512 encoder steps.
            nc.sync.dma_start(w_enc_ih[:], d_enc_wih)
            nc.sync.dma_start(w_enc_hh[:], d_enc_whh)
            nc.vector.memset(ones_col[:], 1.0)
            nc.vector.memset(sstA[:], 0.0)
            nc.vector.memset(sstB[:], 0.0)
            # hT rings: per sample group, per k-half (k01 / k23) so the
            # pk0 recurrent matmuls only wait on the first half of hT.
            # Slot v holds hT of step u with u%DG == v, col k*16+j (2 k each).
            ringA0 = const_pool.tile([128, DG * 32], fp8, tag="ringA0")
            ringA1 = const_pool.tile([128, DG * 32], fp8, tag="ringA1")
            ringB0 = const_pool.tile([128, DG * 32], fp8, tag="ringB0")
            ringB1 = const_pool.tile([128, DG * 32], fp8, tag="ringB1")
            for r in (ringA0, ringA1, ringB0, ringB1):
                nc.vector.memset(r[:], 0.0)
            rings = ((ringA0, ringA1), (ringB0, ringB1))
            ssts = (sstA, sstB)

            def inproj(w_ih, psq, ohq, t, q):
                """Gate-PSUM init, one group: psq col = m*16 + j."""
                for m in range(16):
                    nc.tensor.matmul(
                        psq[:, m * 16:(m + 1) * 16],
                        w_ih[:, m * 128:(m + 1) * 128],
                        ohq[:, t * BS + q * 16: t * BS + q * 16 + 16],
                        start=True, stop=False, skip_group_check=True)

            def rec_mms(w_hh, psq, q, pv):
                """DoubleRow K=256 recurrent matmuls for one sample group.
                pk0 first across all m (it only needs the k01 ring half)."""
                for pk in range(2):
                    hprev = rings[q][pk][:, pv * 32:(pv + 1) * 32]
                    for m in range(16):
                        out = psq[:, m * 16:(m + 1) * 16]
                        w = w_hh[:, (pk * 16 + m) * 256:(pk * 16 + m + 1) * 256]
                        nc.tensor.matmul(
                            out,
                            w.rearrange("p (k c) -> p k c", k=2),
                            hprev.rearrange("p (k j) -> p k j", k=2),
                            start=False, stop=(pk == 1),
                            perf_mode=DR, skip_group_check=True)

            def chain(psqs, v):
                """Both groups' activation chains, stage-interleaved."""
                Ts, tc2s = [], []
                for q in range(2):
                    T = work_pool.tile([128, 256], bf16, tag=f"T{q}",
                                       name=f"T{q}")
                    nc.scalar.activation(T[:], psqs[q][:, :], AF.Tanh,
                                         scale=1.0 / WS)
                    Ts.append(T)
                for q in range(2):
                    T, sst_q = Ts[q], ssts[q]
                    a2 = work_pool.tile([128, 64], f32, tag=f"a2{q}",
                                        name=f"a2{q}")
                    nc.vector.scalar_tensor_tensor(a2[:], T[:, 128:192], 1.0,
                                                   sst_q[:], OP.add, OP.mult)
                    a1 = work_pool.tile([128, 64], f32, tag=f"a1{q}",
                                        name=f"a1{q}")
                    nc.vector.scalar_tensor_tensor(a1[:], T[:, 64:128], 1.0,
                                                   T[:, 0:64], OP.add, OP.mult)
                    nc.vector.scalar_tensor_tensor(sst_q[:], a2[:], 0.5,
                                                   a1[:], OP.mult, OP.add)
                for q in range(2):
                    tc2 = work_pool.tile([128, 64], bf16, tag=f"tc2{q}",
                                         name=f"tc2{q}")
                    nc.scalar.activation(tc2[:], ssts[q][:], AF.Tanh, scale=0.5)
                    tc2s.append(tc2)
                for q in range(2):
                    for h in range(2):
                        nc.vector.scalar_tensor_tensor(
                            rings[q][h][:, v * 32:(v + 1) * 32],
                            Ts[q][:, 192 + h * 32:192 + (h + 1) * 32],
                            1.0, tc2s[q][:, h * 32:(h + 1) * 32],
                            OP.add, OP.mult)

            def dec_tail_mm(g):
                """Batched logits matmuls for steps DG*g .. DG*g+3."""
                ps_l = plog_pool.tile([128, 128], f32, tag="psl")
                for q in range(2):
                    for k in range(4):
                        hsrc = rings[q][k // 2][:, :].rearrange(
                            "p (u k j) -> p u k j", u=DG, k=2, j=16)
                        nc.tensor.matmul(ps_l[:, q * 64:(q + 1) * 64],
                                         w_projt[:, k * 128:(k + 1) * 128],
                                         hsrc[:, :, k % 2, :],
                                         start=(k == 0), stop=(k == 3),
                                         skip_group_check=True)
                return ps_l

            def dec_tail_reduce(ps_l, ohq, gl, accum, g, i_sym):
                """exp / tgt-dot stack (bf16), ones-matmul, accum copy."""
                st = stack_pool.tile([128, 256], bf16, tag="st")
                nc.scalar.activation(st[:, 0:128], ps_l[:, :], AF.Exp,
                                     bias=w_projb[:, 0:1], scale=1.0)
                ohv = ohq[:, :].rearrange("p (t r) -> p t r", r=32)
                for q in range(2):
                    nc.vector.tensor_mul(
                        st[:, 128 + q * 64:128 + (q + 1) * 64]
                        .rearrange("p (v j) -> p v j", v=DG),
                        ps_l[:, q * 64:(q + 1) * 64]
                        .rearrange("p (v j) -> p v j", v=DG),
                        ohv[:, gl * DG:(gl + 1) * DG, q * 16:(q + 1) * 16])
                rp = prsp_pool.tile([1, 256], f32, tag="rp")
                nc.tensor.matmul(rp[:, :], ones_col[:, 0:1], st[:, :],
                                 start=True, stop=True)
                nc.vector.tensor_copy(accum[:, g * 256:(g + 1) * 256],
                                      rp[:, :])

            def body(i, w_hh, w_ih, d_oh, dec):
                ohq = oh_pool.tile([128, U * BS], fp8, tag="oh")
                nc.sync.dma_start(ohq[:], d_oh[:, bass.ts(i, U * BS)])
                accum = None
                if dec:
                    accum = acc_pool.tile([1, (U // DG) * 256], f32, tag="accum")
                psA = pgA_pool.tile([128, 256], f32, tag="psA")
                psB = pgB_pool.tile([128, 256], f32, tag="psB")
                inproj(w_ih, psA, ohq, 0, 0)
                inproj(w_ih, psB, ohq, 0, 1)
                pend_mm = None
                pend_red = None
                for u in range(U):
                    g, v = u // DG, u % DG
                    pv = (u - 1) % DG
                    psqs = (psA, psB)
                    rec_mms(w_hh, psA, 0, pv)
                    rec_mms(w_hh, psB, 1, pv)
                    # proj matmuls for the previous dec group run here: after
                    # this step's recurrent matmuls (so they don't delay them
                    # in the PE queue) but before this step's hT overwrites
                    # ring slot 0.
                    if pend_mm is not None and not _ABL_TAIL:
                        pend_red = (dec_tail_mm(pend_mm), ohq, pend_mm,
                                    accum, pend_mm, i)
                        pend_mm = None
                    pend_mm = None if _ABL_TAIL else pend_mm
                    if u + 1 < U:
                        psA_n = pgA_pool.tile([128, 256], f32, tag="psA")
                        psB_n = pgB_pool.tile([128, 256], f32, tag="psB")
                        inproj(w_ih, psA_n, ohq, u + 1, 0)
                        inproj(w_ih, psB_n, ohq, u + 1, 1)
                    chain(psqs, v)
                    if pend_red is not None:
                        dec_tail_reduce(*pend_red)
                        pend_red = None
                    if dec and v == DG - 1:
                        pend_mm = g
                    if u + 1 < U:
                        psA, psB = psA_n, psB_n
                if pend_mm is not None:
                    dec_tail_reduce(dec_tail_mm(pend_mm), ohq, pend_mm,
                                    accum, pend_mm, i)
                if dec:
                    nc.sync.dma_start(d_res[:, bass.ts(i, (U // DG) * 256)],
                                      accum[:])

            if unrolled:
                for i in range(n_iters):
                    body(i, w_enc_hh, w_enc_ih, d_enc_oh, False)
                nc.sync.dma_start(w_dec_ih[:], d_dec_wih)
                nc.sync.dma_start(w_dec_hh[:], d_dec_whh)
                nc.sync.dma_start(w_projt[:], d_projt)
                nc.sync.dma_start(w_projb[:], d_projb)
                for i in range(n_iters):
                    body(i, w_dec_hh, w_dec_ih, d_dec_oh, True)
            else:
                with tc.For_i(0, n_iters, 1, hint_engines=(PE,), name="enc") as i:
                    body(i, w_enc_hh, w_enc_ih, d_enc_oh, False)
                nc.sync.dma_start(w_dec_ih[:], d_dec_wih)
                nc.sync.dma_start(w_dec_hh[:], d_dec_whh)
                nc.sync.dma_start(w_projt[:], d_projt)
                nc.sync.dma_start(w_projb[:], d_projb)
                with tc.For_i(0, n_iters, 1, hint_engines=(PE,), name="dec") as i:
                    body(i, w_dec_hh, w_dec_ih, d_dec_oh, True)

    nc.compile()
    return nc


def _run(inputs, n_steps=S, trace=False):
    from concourse import bass_utils

    key = n_steps
    if key not in _CACHE:
        _CACHE[key] = _build_module(n_steps)
    nc = _CACHE[key]

    enc_wih, enc_whh = _prep_weights(inputs["enc_W_ih"], inputs["enc_W_hh"],
                                     inputs["enc_b_ih"], inputs["enc_b_hh"])
    dec_wih, dec_whh = _prep_weights(inputs["dec_W_ih"], inputs["dec_W_hh"],
                                     inputs["dec_b_ih"], inputs["dec_b_hh"])
    import ml_dtypes
    projW = 0.5 * np.asarray(inputs["proj_W"], np.float64)  # [128, 512]
    projt = (np.ascontiguousarray(projW.T).reshape(4, 128, 128)
             .transpose(1, 0, 2).reshape(128, 512).astype(ml_dtypes.bfloat16))
    projb = np.ascontiguousarray(
        np.asarray(inputs["proj_b"], np.float32).reshape(128, 1))

    C_idx = np.asarray(inputs["C_idx"])[:, :n_steps]
    E = np.asarray(inputs["E"])
    Etgt = E[:, :n_steps]

    in_maps = []
    for c in range(NCORES):
        sl = slice(c * BS, (c + 1) * BS)
        in_maps.append({
            "enc_whh": enc_whh, "enc_wih": enc_wih,
            "dec_whh": dec_whh, "dec_wih": dec_wih,
            "projt": np.ascontiguousarray(projt), "projb": projb,
            "enc_oh": _onehot_stream(C_idx[sl]),
            "dec_oh": _onehot_stream(Etgt[sl]),
        })

    res = bass_utils.run_bass_kernel_spmd(
        nc, in_maps, core_ids=list(range(NCORES)), trace=trace,
        trace_cores=[0] if trace else None)

    # ---- host-side loss assembly (float64) ----
    proj_b = np.asarray(inputs["proj_b"], np.float64)
    nll = np.empty((B, n_steps), np.float64)
    for c in range(NCORES):
        r = np.asarray(res.results[c]["res"], np.float64).reshape(
            n_steps // DG, 2, 2, DG, 16)       # [g, {sumexp,tgtdot}, q, v, j]
        r = r.transpose(0, 1, 3, 2, 4)         # [g, s, v, q, j]
        sumexp = r[:, 0].reshape(n_steps, BS)  # [u, sample 16q+j]
        tgtdot = r[:, 1].reshape(n_steps, BS)
        tgt = Etgt[c * BS:(c + 1) * BS]            # [j, u]
        bias_t = proj_b[tgt]                       # [j, u]
        nll[c * BS:(c + 1) * BS] = (np.log(sumexp).T
                                    - (tgtdot.T + bias_t))
    mask = (Etgt != 0).astype(np.float64)          # [B, u]
    num = (nll * mask).sum(axis=0)
    cnt = mask.sum(axis=0)
    step_loss = np.where(cnt > 0, num / np.maximum(cnt, 1.0), 0.0)
    total = np.float32(step_loss.sum())
    return total, res


def kernel(**inputs) -> np.ndarray:
    total, _ = _run(inputs, n_steps=S,
                    trace=bool(int(os.environ.get("LSTM_TRACE", "0"))))
    return np.float32(total)



# revision 3
# speedup vs baseline: 1.0565x; 1.0565x over previous
"""AutoCompleteDecoderModel (LSTM enc-dec + CE loss) on 8 Trainium2 cores.

v3 strategy (B=256, S=512, H=512, V=128; 8 cores x 32 samples):
 - gates.T [2048, 32] per step in PSUM; gate m-tile order [g|i|f|o].
 - 2-step PSUM flights [128, 512] per sample group: the input projection
   (one-hot, bias folded) for both steps of a flight is ONE matmul per
   m-tile (N=32), issued at the start of the flight's first step so the
   independent PE work fills the stall where the PE waits on the previous
   chain's ring write.
 - Recurrent matmuls use fp8e4 weights in DoubleRow mode: K=256 per
   instruction -> 32 matmuls/step instead of 64.  h state (hT=2h) is
   stored fp8e4; weights are scaled x64 (and i/f/o rows by an extra 0.5 so
   all four gates use a single tanh scale), unscaled in the ACT pre-scale.
 - Samples split into two anti-phase groups of 16: each group's activation
   chain (1 tanh [128,256], 3 STT, tanh(c), STT) overlaps the other group's
   matmuls, hiding the chain latency behind the recurrence of the peer.
 - c-state (sst = 2c) and chain intermediates in bf16: all-16-bit operands
   let the DVE scalar_tensor_tensor ops run in 2x_1P packed mode.
 - Decoder tail batched over groups of DG=8 steps: logits.T via 8 matmuls
   (N=128) reading an hT ring, one exp [128,256], tgt-dot muls, one
   ones-matmul reduce [1,512], one copy into the result accumulator.
 - U=256 steps per hw-loop iteration (2 For_i iterations per phase) to
   minimize loop back-edge resyncs; enc W_hh DMA split in halves so the
   first recurrent matmuls start sooner.
 - Host: nll = ln(sumexp) - (tgtdot + proj_b[tgt]), masked mean, sum.
"""

import os
import sys

import numpy as np

if "/opt/trn_rl_repo" not in sys.path:
    sys.path.insert(0, "/opt/trn_rl_repo")

B, S, H, V = 256, 512, 512, 128
NCORES = 8
BS = B // NCORES   # 32 samples per core
U = int(os.environ.get("LSTM_U", "256"))  # steps per hw-loop iteration
FS = 2             # steps per PSUM flight
DG = int(os.environ.get("LSTM_DG", "8"))  # steps per decoder tail group
WS = 64.0          # fp8 weight scale

_CACHE = {}

_PERM = None


def _perm():
    global _PERM
    if _PERM is None:
        _PERM = np.concatenate([
            np.arange(1024, 1536),  # g
            np.arange(0, 512),      # i
            np.arange(512, 1024),   # f
            np.arange(1536, 2048),  # o
        ])
    return _PERM


def _prep_weights(W_ih, W_hh, b_ih, b_hh):
    """Fold biases into W_ih, fold the hT=2h and single-tanh-scale factors,
    scale x64, quantize fp8e4, and pack for the kernel layouts."""
    import ml_dtypes

    fp8 = ml_dtypes.float8_e4m3
    perm = _perm()
    Wi = (np.asarray(W_ih, np.float64) + (np.asarray(b_ih, np.float64)
          + np.asarray(b_hh, np.float64))[:, None])[perm]  # [2048, 128]
    Wh = 0.5 * np.asarray(W_hh, np.float64)[perm]          # [2048, 512]
    Wi[512:] *= 0.5  # i,f,o rows: single tanh scale (tanh(z/2))
    Wh[512:] *= 0.5
    Wi *= WS
    Wh *= WS
    # input proj lhsT tiles: wih_t [V=128, 2048], m-tile m at cols m*128
    wih_t = np.ascontiguousarray(Wi.T).astype(fp8)
    # recurrent DoubleRow pairs: whh_dr [128, 8192], block (pk, m) at col
    # (pk*16+m)*256, within block [kk=2, c=128]; k-tile = 2*pk+kk
    Wt = np.ascontiguousarray(Wh.T)                        # [512, 2048]
    whh_dr = (Wt.reshape(2, 2, 128, 16, 128)               # [pk,kk,p,m,c]
              .transpose(2, 0, 3, 1, 4)                    # [p,pk,m,kk,c]
              .reshape(128, 8192).astype(fp8))
    return np.ascontiguousarray(wih_t), np.ascontiguousarray(whh_dr)


def _onehot_stream(idx):
    """idx [BS, S] int -> [128, S*32] fp8e4, col t*32+j = (idx[j,t]==v)."""
    import ml_dtypes
    oh = (np.arange(V, dtype=np.int32)[:, None, None]
          == np.asarray(idx, np.int32).T[None, :, :])  # [V, S, BS]
    return np.ascontiguousarray(
        oh.reshape(V, -1).astype(ml_dtypes.float8_e4m3))


def _build_module(n_steps, unrolled=False):
    _ABL_TAIL = bool(int(os.environ.get("ABL_TAIL", "0")))
    _FLIGHT_INPROJ = bool(int(os.environ.get("FLIGHT_INPROJ", "1")))
    _POOL_A1 = bool(int(os.environ.get("POOL_A1", "0")))
    _POOL_RING = bool(int(os.environ.get("POOL_RING", "0")))
    _SST_BF16 = bool(int(os.environ.get("SST_BF16", "1")))
    import concourse.bacc as bacc
    import concourse.bass as bass
    import concourse.mybir as mybir
    import concourse.tile as tile

    f32 = mybir.dt.float32
    bf16 = mybir.dt.bfloat16
    fp8 = mybir.dt.float8e4
    AF = mybir.ActivationFunctionType
    OP = mybir.AluOpType
    PE = mybir.EngineType.PE
    DR = mybir.MatmulPerfMode.DoubleRow

    assert n_steps % U == 0
    n_iters = n_steps // U

    nc = bacc.Bacc("TRN2", target_bir_lowering=False, debug=False,
                   num_devices=NCORES)

    d_enc_whh = nc.dram_tensor("enc_whh", [128, 8192], fp8, kind="ExternalInput").ap()
    d_enc_wih = nc.dram_tensor("enc_wih", [128, 2048], fp8, kind="ExternalInput").ap()
    d_dec_whh = nc.dram_tensor("dec_whh", [128, 8192], fp8, kind="ExternalInput").ap()
    d_dec_wih = nc.dram_tensor("dec_wih", [128, 2048], fp8, kind="ExternalInput").ap()
    d_projt = nc.dram_tensor("projt", [128, 512], bf16, kind="ExternalInput").ap()
    d_projb = nc.dram_tensor("projb", [128, 1], f32, kind="ExternalInput").ap()
    d_enc_oh = nc.dram_tensor("enc_oh", [128, n_steps * BS], fp8, kind="ExternalInput").ap()
    d_dec_oh = nc.dram_tensor("dec_oh", [128, n_steps * BS], fp8, kind="ExternalInput").ap()
    d_res = nc.dram_tensor("res", [1, n_steps * 64], f32, kind="ExternalOutput").ap()

    with tile.TileContext(nc) as tc:
        with (
            tc.tile_pool(name="const", bufs=1) as const_pool,
            tc.tile_pool(name="oh", bufs=3) as oh_pool,
            tc.tile_pool(name="pgA", bufs=2 if _FLIGHT_INPROJ else 3,
                         space="PSUM") as pgA_pool,
            tc.tile_pool(name="pgB", bufs=2 if _FLIGHT_INPROJ else 3,
                         space="PSUM") as pgB_pool,
            tc.tile_pool(name="plog", bufs=1, space="PSUM") as plog_pool,
            tc.tile_pool(name="prsp", bufs=1, space="PSUM") as prsp_pool,
            tc.tile_pool(name="work", bufs=4) as work_pool,
            tc.tile_pool(name="stack", bufs=2) as stack_pool,
            tc.tile_pool(name="acc", bufs=2) as acc_pool,
        ):
            w_enc_hh = const_pool.tile([128, 8192], fp8, tag="wehh")
            w_enc_ih = const_pool.tile([128, 2048], fp8, tag="weih")
            w_dec_hh = const_pool.tile([128, 8192], fp8, tag="wdhh")
            w_dec_ih = const_pool.tile([128, 2048], fp8, tag="wdih")
            w_projt = const_pool.tile([128, 512], bf16, tag="wpt")
            w_projb = const_pool.tile([128, 1], f32, tag="wpb")
            ones_col = const_pool.tile([128, 1], bf16, tag="ones")
            sdt = bf16 if _SST_BF16 else f32
            sstA = const_pool.tile([128, 64], sdt, tag="sstA")
            sstB = const_pool.tile([128, 64], sdt, tag="sstB")

            # Encoder weights first (the first steps need wih then whh);
            # decoder weights + projection load after the encoder loop is
            # issued, overlapping the 512 encoder steps.
            nc.sync.dma_start(w_enc_ih[:], d_enc_wih)
            # pk0 half first: the first rec matmuls only need cols 0..4095
            nc.sync.dma_start(w_enc_hh[:, 0:4096], d_enc_whh[:, 0:4096])
            nc.sync.dma_start(w_enc_hh[:, 4096:8192], d_enc_whh[:, 4096:8192])
            nc.vector.memset(ones_col[:], 1.0)
            nc.vector.memset(sstA[:], 0.0)
            nc.vector.memset(sstB[:], 0.0)
            # hT rings: per sample group, per k-half (k01 / k23) so the
            # pk0 recurrent matmuls only wait on the first half of hT.
            # Slot v holds hT of step u with u%DG == v, col k*16+j (2 k each).
            ringA0 = const_pool.tile([128, DG * 32], fp8, tag="ringA0")
            ringA1 = const_pool.tile([128, DG * 32], fp8, tag="ringA1")
            ringB0 = const_pool.tile([128, DG * 32], fp8, tag="ringB0")
            ringB1 = const_pool.tile([128, DG * 32], fp8, tag="ringB1")
            for r in (ringA0, ringA1, ringB0, ringB1):
                nc.vector.memset(r[:], 0.0)
            rings = ((ringA0, ringA1), (ringB0, ringB1))
            ssts = (sstA, sstB)

            def inproj(w_ih, psq, ohq, t, q):
                """Gate-PSUM init, one group: psq col = m*16 + j."""
                for m in range(16):
                    nc.tensor.matmul(
                        psq[:, m * 16:(m + 1) * 16],
                        w_ih[:, m * 128:(m + 1) * 128],
                        ohq[:, t * BS + q * 16: t * BS + q * 16 + 16],
                        start=True, stop=False, skip_group_check=True)

            def inproj2(w_ih, psf, ohq, t, q):
                """Gate-PSUM init for a 2-step flight (steps t, t+1), one
                group: psf [128, 512], col s*256 + m*16 + j."""
                ps3 = psf[:, :].rearrange("p (s mj) -> p s mj", s=2)
                oh4 = ohq[:, :].rearrange("p (f s q j) -> p f s q j",
                                          s=2, q=2, j=16)
                for m in range(16):
                    nc.tensor.matmul(
                        ps3[:, :, m * 16:(m + 1) * 16],
                        w_ih[:, m * 128:(m + 1) * 128],
                        oh4[:, t // 2, :, q, :],
                        start=True, stop=False, skip_group_check=True)

            def rec_mms(w_hh, psq, q, pv):
                """DoubleRow K=256 recurrent matmuls for one sample group.
                pk0 first across all m (it only needs the k01 ring half)."""
                for pk in range(2):
                    hprev = rings[q][pk][:, pv * 32:(pv + 1) * 32]
                    for m in range(16):
                        out = psq[:, m * 16:(m + 1) * 16]
                        w = w_hh[:, (pk * 16 + m) * 256:(pk * 16 + m + 1) * 256]
                        nc.tensor.matmul(
                            out,
                            w.rearrange("p (k c) -> p k c", k=2),
                            hprev.rearrange("p (k j) -> p k j", k=2),
                            start=False, stop=(pk == 1),
                            perf_mode=DR, skip_group_check=True)

            def chain(psqs, v):
                """Both groups' activation chains, stage-interleaved."""
                Ts, tc2s = [], []
                for q in range(2):
                    T = work_pool.tile([128, 256], bf16, tag=f"T{q}",
                                       name=f"T{q}")
                    nc.scalar.activation(T[:], psqs[q][:, :], AF.Tanh,
                                         scale=1.0 / WS)
                    Ts.append(T)
                a1eng = nc.gpsimd if _POOL_A1 else nc.vector
                a1s = []
                for q in range(2):
                    # a1 is off the c-critical path: compute it on Pool so
                    # the DVE a2->sst chain and Pool a1 run concurrently.
                    a1 = work_pool.tile([128, 64], f32, tag=f"a1{q}",
                                        name=f"a1{q}")
                    a1eng.scalar_tensor_tensor(a1[:], Ts[q][:, 64:128], 1.0,
                                               Ts[q][:, 0:64], OP.add,
                                               OP.mult)
                    a1s.append(a1)
                for q in range(2):
                    T, sst_q = Ts[q], ssts[q]
                    a2 = work_pool.tile([128, 64], f32, tag=f"a2{q}",
                                        name=f"a2{q}")
                    nc.vector.scalar_tensor_tensor(a2[:], T[:, 128:192], 1.0,
                                                   sst_q[:], OP.add, OP.mult)
                    nc.vector.scalar_tensor_tensor(sst_q[:], a2[:], 0.5,
                                                   a1s[q][:], OP.mult, OP.add)
                for q in range(2):
                    tc2 = work_pool.tile([128, 64], bf16, tag=f"tc2{q}",
                                         name=f"tc2{q}")
                    nc.scalar.activation(tc2[:], ssts[q][:], AF.Tanh, scale=0.5)
                    tc2s.append(tc2)
                for q in range(2):
                    for h in range(2):
                        nc.vector.scalar_tensor_tensor(
                            rings[q][h][:, v * 32:(v + 1) * 32],
                            Ts[q][:, 192 + h * 32:192 + (h + 1) * 32],
                            1.0, tc2s[q][:, h * 32:(h + 1) * 32],
                            OP.add, OP.mult)

            DGW = DG * 16  # per-group tail width

            def dec_tail_mm(g):
                """Batched logits matmuls for steps DG*g .. DG*g+DG-1."""
                ps_l = plog_pool.tile([128, 2 * DGW], f32, tag="psl")
                for q in range(2):
                    for k in range(4):
                        hsrc = rings[q][k // 2][:, :].rearrange(
                            "p (u k j) -> p u k j", u=DG, k=2, j=16)
                        nc.tensor.matmul(ps_l[:, q * DGW:(q + 1) * DGW],
                                         w_projt[:, k * 128:(k + 1) * 128],
                                         hsrc[:, :, k % 2, :],
                                         start=(k == 0), stop=(k == 3),
                                         skip_group_check=True)
                return ps_l

            def dec_tail_reduce(ps_l, ohq, gl, accum, g, i_sym):
                """exp / tgt-dot stack (bf16), ones-matmul, accum copy."""
                st = stack_pool.tile([128, 4 * DGW], bf16, tag="st")
                nc.scalar.activation(st[:, 0:2 * DGW], ps_l[:, :], AF.Exp,
                                     bias=w_projb[:, 0:1], scale=1.0)
                ohv = ohq[:, :].rearrange("p (t r) -> p t r", r=32)
                for q in range(2):
                    nc.vector.tensor_mul(
                        st[:, 2 * DGW + q * DGW:2 * DGW + (q + 1) * DGW]
                        .rearrange("p (v j) -> p v j", v=DG),
                        ps_l[:, q * DGW:(q + 1) * DGW]
                        .rearrange("p (v j) -> p v j", v=DG),
                        ohv[:, gl * DG:(gl + 1) * DG, q * 16:(q + 1) * 16])
                rp = prsp_pool.tile([1, 4 * DGW], f32, tag="rp")
                nc.tensor.matmul(rp[:, :], ones_col[:, 0:1], st[:, :],
                                 start=True, stop=True)
                nc.vector.tensor_copy(accum[:, g * 4 * DGW:(g + 1) * 4 * DGW],
                                      rp[:, :])

            def body(i, w_hh, w_ih, d_oh, dec):
                ohq = oh_pool.tile([128, U * BS], fp8, tag="oh")
                nc.sync.dma_start(ohq[:], d_oh[:, bass.ts(i, U * BS)])
                accum = None
                if dec:
                    accum = acc_pool.tile([1, U * 64], f32, tag="accum")
                if _FLIGHT_INPROJ:
                    psA = pgA_pool.tile([128, 512], f32, tag="psA")
                    psB = pgB_pool.tile([128, 512], f32, tag="psB")
                    inproj2(w_ih, psA, ohq, 0, 0)
                    inproj2(w_ih, psB, ohq, 0, 1)
                else:
                    psA = pgA_pool.tile([128, 256], f32, tag="psA")
                    psB = pgB_pool.tile([128, 256], f32, tag="psB")
                    inproj(w_ih, psA, ohq, 0, 0)
                    inproj(w_ih, psB, ohq, 0, 1)
                pend_mm = None
                pend_red = None
                for u in range(U):
                    g, v = u // DG, u % DG
                    pv = (u - 1) % DG
                    if _FLIGHT_INPROJ:
                        # Issue the new flight's input projections at the
                        # start of its first step: independent PE work sits
                        # in the FIFO exactly where the PE would otherwise
                        # stall waiting for the previous chain's ring write.
                        s = u % 2
                        if s == 0 and u > 0:
                            psA = pgA_pool.tile([128, 512], f32, tag="psA")
                            psB = pgB_pool.tile([128, 512], f32, tag="psB")
                            inproj2(w_ih, psA, ohq, u, 0)
                            inproj2(w_ih, psB, ohq, u, 1)
                        psqA = psA[:, s * 256:(s + 1) * 256]
                        psqB = psB[:, s * 256:(s + 1) * 256]
                    else:
                        psqA, psqB = psA, psB
                    psqs = (psqA, psqB)
                    rec_mms(w_hh, psqA, 0, pv)
                    rec_mms(w_hh, psqB, 1, pv)
                    # proj matmuls for the previous dec group run here: after
                    # this step's recurrent matmuls (so they don't delay them
                    # in the PE queue) but before this step's hT overwrites
                    # ring slot 0.
                    if pend_mm is not None and not _ABL_TAIL:
                        pend_red = (dec_tail_mm(pend_mm), ohq, pend_mm,
                                    accum, pend_mm, i)
                        pend_mm = None
                    pend_mm = None if _ABL_TAIL else pend_mm
                    if not _FLIGHT_INPROJ and u + 1 < U:
                        psA_n = pgA_pool.tile([128, 256], f32, tag="psA")
                        psB_n = pgB_pool.tile([128, 256], f32, tag="psB")
                        inproj(w_ih, psA_n, ohq, u + 1, 0)
                        inproj(w_ih, psB_n, ohq, u + 1, 1)
                    chain(psqs, v)
                    if pend_red is not None:
                        dec_tail_reduce(*pend_red)
                        pend_red = None
                    if dec and v == DG - 1:
                        pend_mm = g
                    if not _FLIGHT_INPROJ and u + 1 < U:
                        psA, psB = psA_n, psB_n
                if pend_mm is not None:
                    dec_tail_reduce(dec_tail_mm(pend_mm), ohq, pend_mm,
                                    accum, pend_mm, i)
                if dec:
                    nc.sync.dma_start(d_res[:, bass.ts(i, U * 64)],
                                      accum[:])

            _REPEAT = int(os.environ.get("REPEAT_MODULE", "1"))

            def phases():
                with tc.For_i(0, n_iters, 1, hint_engines=(PE,), name="enc") as i:
                    body(i, w_enc_hh, w_enc_ih, d_enc_oh, False)
                nc.sync.dma_start(w_dec_ih[:], d_dec_wih)
                nc.sync.dma_start(w_dec_hh[:], d_dec_whh)
                nc.sync.dma_start(w_projt[:], d_projt)
                nc.sync.dma_start(w_projb[:], d_projb)
                with tc.For_i(0, n_iters, 1, hint_engines=(PE,), name="dec") as i:
                    body(i, w_dec_hh, w_dec_ih, d_dec_oh, True)

            if unrolled:
                for i in range(n_iters):
                    body(i, w_enc_hh, w_enc_ih, d_enc_oh, False)
                nc.sync.dma_start(w_dec_ih[:], d_dec_wih)
                nc.sync.dma_start(w_dec_hh[:], d_dec_whh)
                nc.sync.dma_start(w_projt[:], d_projt)
                nc.sync.dma_start(w_projb[:], d_projb)
                for i in range(n_iters):
                    body(i, w_dec_hh, w_dec_ih, d_dec_oh, True)
            elif _REPEAT > 1:
                with tc.For_i(0, _REPEAT, 1, name="rep"):
                    phases()
            else:
                phases()

    nc.compile()
    return nc


def _prep_inmaps(inputs, n_steps=S):
    enc_wih, enc_whh = _prep_weights(inputs["enc_W_ih"], inputs["enc_W_hh"],
                                     inputs["enc_b_ih"], inputs["enc_b_hh"])
    dec_wih, dec_whh = _prep_weights(inputs["dec_W_ih"], inputs["dec_W_hh"],
                                     inputs["dec_b_ih"], inputs["dec_b_hh"])
    import ml_dtypes
    projW = 0.5 * np.asarray(inputs["proj_W"], np.float64)  # [128, 512]
    projt = (np.ascontiguousarray(projW.T).reshape(4, 128, 128)
             .transpose(1, 0, 2).reshape(128, 512).astype(ml_dtypes.bfloat16))
    projb = np.ascontiguousarray(
        np.asarray(inputs["proj_b"], np.float32).reshape(128, 1))

    C_idx = np.asarray(inputs["C_idx"])[:, :n_steps]
    Etgt = np.asarray(inputs["E"])[:, :n_steps]

    in_maps = []
    for c in range(NCORES):
        sl = slice(c * BS, (c + 1) * BS)
        in_maps.append({
            "enc_whh": enc_whh, "enc_wih": enc_wih,
            "dec_whh": dec_whh, "dec_wih": dec_wih,
            "projt": np.ascontiguousarray(projt), "projb": projb,
            "enc_oh": _onehot_stream(C_idx[sl]),
            "dec_oh": _onehot_stream(Etgt[sl]),
        })
    return in_maps


def _assemble(inputs, results, n_steps=S):
    """results: list of per-core {'res': ...} dicts."""
    Etgt = np.asarray(inputs["E"])[:, :n_steps]
    proj_b = np.asarray(inputs["proj_b"], np.float64)
    nll = np.empty((B, n_steps), np.float64)
    for c in range(NCORES):
        r = np.asarray(results[c]["res"], np.float64).reshape(
            n_steps // DG, 2, 2, DG, 16)       # [g, {sumexp,tgtdot}, q, v, j]
        r = r.transpose(0, 1, 3, 2, 4)         # [g, s, v, q, j]
        sumexp = r[:, 0].reshape(n_steps, BS)  # [u, sample 16q+j]
        tgtdot = r[:, 1].reshape(n_steps, BS)
        tgt = Etgt[c * BS:(c + 1) * BS]            # [j, u]
        bias_t = proj_b[tgt]                       # [j, u]
        nll[c * BS:(c + 1) * BS] = (np.log(sumexp).T
                                    - (tgtdot.T + bias_t))
    mask = (Etgt != 0).astype(np.float64)          # [B, u]
    num = (nll * mask).sum(axis=0)
    cnt = mask.sum(axis=0)
    step_loss = np.where(cnt > 0, num / np.maximum(cnt, 1.0), 0.0)
    return np.float32(step_loss.sum())


def _run(inputs, n_steps=S, trace=False):
    from concourse import bass_utils

    key = n_steps
    if key not in _CACHE:
        _CACHE[key] = _build_module(n_steps)
    nc = _CACHE[key]

    in_maps = _prep_inmaps(inputs, n_steps)
    res = bass_utils.run_bass_kernel_spmd(
        nc, in_maps, core_ids=list(range(NCORES)), trace=trace,
        trace_cores=[0] if trace else None)
    total = _assemble(inputs, res.results, n_steps)
    return total, res


def kernel(**inputs) -> np.ndarray:
    total, _ = _run(inputs, n_steps=S,
                    trace=bool(int(os.environ.get("LSTM_TRACE", "0"))))
    return np.float32(total)



# revision 4
# speedup vs baseline: 1.0625x; 1.0056x over previous
"""AutoCompleteDecoderModel (LSTM enc-dec + CE loss) on 8 Trainium2 cores.

v3 strategy (B=256, S=512, H=512, V=128; 8 cores x 32 samples):
 - gates.T [2048, 32] per step in PSUM; gate m-tile order [g|i|f|o].
 - 2-step PSUM flights [128, 512] per sample group: the input projection
   (one-hot, bias folded) for both steps of a flight is ONE matmul per
   m-tile (N=32), issued at the start of the flight's first step so the
   independent PE work fills the stall where the PE waits on the previous
   chain's ring write.
 - Recurrent matmuls use fp8e4 weights in DoubleRow mode: K=256 per
   instruction -> 32 matmuls/step instead of 64.  h state (hT=2h) is
   stored fp8e4; weights are scaled x64 (and i/f/o rows by an extra 0.5 so
   all four gates use a single tanh scale), unscaled in the ACT pre-scale.
 - Samples split into two anti-phase groups of 16: each group's activation
   chain (1 tanh [128,256], 3 STT, tanh(c), STT) overlaps the other group's
   matmuls, hiding the chain latency behind the recurrence of the peer.
 - c-state (sst = 2c) and chain intermediates in bf16: all-16-bit operands
   let the DVE scalar_tensor_tensor ops run in 2x_1P packed mode.
 - Decoder tail batched over groups of DG=4 steps: logits.T via 8 matmuls
   (N=64) reading an hT ring, one exp [128,128], tgt-dot muls, one
   ones-matmul reduce [1,256], one copy into the result accumulator.
 - U=256 steps per hw-loop iteration (2 For_i iterations per phase) to
   minimize loop back-edge resyncs; enc W_hh DMA split in halves so the
   first recurrent matmuls start sooner.
 - Host: nll = ln(sumexp) - (tgtdot + proj_b[tgt]), masked mean, sum.
"""

import os
import sys

import numpy as np

if "/opt/trn_rl_repo" not in sys.path:
    sys.path.insert(0, "/opt/trn_rl_repo")

B, S, H, V = 256, 512, 512, 128
NCORES = 8
BS = B // NCORES   # 32 samples per core
U = int(os.environ.get("LSTM_U", "256"))  # steps per hw-loop iteration
FS = 2             # steps per PSUM flight
DG = int(os.environ.get("LSTM_DG", "4"))  # steps per decoder tail group
WS = 64.0          # fp8 weight scale

_CACHE = {}

_PERM = None


def _perm():
    global _PERM
    if _PERM is None:
        _PERM = np.concatenate([
            np.arange(1024, 1536),  # g
            np.arange(0, 512),      # i
            np.arange(512, 1024),   # f
            np.arange(1536, 2048),  # o
        ])
    return _PERM


def _prep_weights(W_ih, W_hh, b_ih, b_hh):
    """Fold biases into W_ih, fold the hT=2h and single-tanh-scale factors,
    scale x64, quantize fp8e4, and pack for the kernel layouts."""
    import ml_dtypes

    fp8 = ml_dtypes.float8_e4m3
    perm = _perm()
    Wi = (np.asarray(W_ih, np.float64) + (np.asarray(b_ih, np.float64)
          + np.asarray(b_hh, np.float64))[:, None])[perm]  # [2048, 128]
    Wh = 0.5 * np.asarray(W_hh, np.float64)[perm]          # [2048, 512]
    Wi[512:] *= 0.5  # i,f,o rows: single tanh scale (tanh(z/2))
    Wh[512:] *= 0.5
    Wi *= WS
    Wh *= WS
    # input proj lhsT tiles: wih_t [V=128, 2048], m-tile m at cols m*128
    wih_t = np.ascontiguousarray(Wi.T).astype(fp8)
    # recurrent DoubleRow pairs: whh_dr [128, 8192], block (pk, m) at col
    # (pk*16+m)*256, within block [kk=2, c=128]; k-tile = 2*pk+kk
    Wt = np.ascontiguousarray(Wh.T)                        # [512, 2048]
    whh_dr = (Wt.reshape(2, 2, 128, 16, 128)               # [pk,kk,p,m,c]
              .transpose(2, 0, 3, 1, 4)                    # [p,pk,m,kk,c]
              .reshape(128, 8192).astype(fp8))
    return np.ascontiguousarray(wih_t), np.ascontiguousarray(whh_dr)


def _onehot_stream(idx):
    """idx [BS, S] int -> [128, S*32] fp8e4, col t*32+j = (idx[j,t]==v)."""
    import ml_dtypes
    oh = (np.arange(V, dtype=np.int32)[:, None, None]
          == np.asarray(idx, np.int32).T[None, :, :])  # [V, S, BS]
    return np.ascontiguousarray(
        oh.reshape(V, -1).astype(ml_dtypes.float8_e4m3))


def _build_module(n_steps, unrolled=False):
    _ABL_TAIL = bool(int(os.environ.get("ABL_TAIL", "0")))
    _FLIGHT_INPROJ = bool(int(os.environ.get("FLIGHT_INPROJ", "1")))
    _POOL_A1 = bool(int(os.environ.get("POOL_A1", "0")))
    _POOL_RING = bool(int(os.environ.get("POOL_RING", "0")))
    _SST_BF16 = bool(int(os.environ.get("SST_BF16", "1")))
    import concourse.bacc as bacc
    import concourse.bass as bass
    import concourse.mybir as mybir
    import concourse.tile as tile

    f32 = mybir.dt.float32
    bf16 = mybir.dt.bfloat16
    fp8 = mybir.dt.float8e4
    AF = mybir.ActivationFunctionType
    OP = mybir.AluOpType
    PE = mybir.EngineType.PE
    DR = mybir.MatmulPerfMode.DoubleRow

    assert n_steps % U == 0
    n_iters = n_steps // U

    nc = bacc.Bacc("TRN2", target_bir_lowering=False, debug=False,
                   num_devices=NCORES)

    d_enc_whh = nc.dram_tensor("enc_whh", [128, 8192], fp8, kind="ExternalInput").ap()
    d_enc_wih = nc.dram_tensor("enc_wih", [128, 2048], fp8, kind="ExternalInput").ap()
    d_dec_whh = nc.dram_tensor("dec_whh", [128, 8192], fp8, kind="ExternalInput").ap()
    d_dec_wih = nc.dram_tensor("dec_wih", [128, 2048], fp8, kind="ExternalInput").ap()
    d_projt = nc.dram_tensor("projt", [128, 512], bf16, kind="ExternalInput").ap()
    d_projb = nc.dram_tensor("projb", [128, 1], f32, kind="ExternalInput").ap()
    d_enc_oh = nc.dram_tensor("enc_oh", [128, n_steps * BS], fp8, kind="ExternalInput").ap()
    d_dec_oh = nc.dram_tensor("dec_oh", [128, n_steps * BS], fp8, kind="ExternalInput").ap()
    d_res = nc.dram_tensor("res", [1, n_steps * 64], f32, kind="ExternalOutput").ap()

    with tile.TileContext(nc) as tc:
        with (
            tc.tile_pool(name="const", bufs=1) as const_pool,
            tc.tile_pool(name="oh", bufs=3) as oh_pool,
            tc.tile_pool(name="pgA", bufs=2 if _FLIGHT_INPROJ else 3,
                         space="PSUM") as pgA_pool,
            tc.tile_pool(name="pgB", bufs=2 if _FLIGHT_INPROJ else 3,
                         space="PSUM") as pgB_pool,
            tc.tile_pool(name="plog", bufs=1, space="PSUM") as plog_pool,
            tc.tile_pool(name="prsp", bufs=1, space="PSUM") as prsp_pool,
            tc.tile_pool(name="work", bufs=4) as work_pool,
            tc.tile_pool(name="stack", bufs=2) as stack_pool,
            tc.tile_pool(name="acc", bufs=2) as acc_pool,
        ):
            w_enc_hh = const_pool.tile([128, 8192], fp8, tag="wehh")
            w_enc_ih = const_pool.tile([128, 2048], fp8, tag="weih")
            w_dec_hh = const_pool.tile([128, 8192], fp8, tag="wdhh")
            w_dec_ih = const_pool.tile([128, 2048], fp8, tag="wdih")
            w_projt = const_pool.tile([128, 512], bf16, tag="wpt")
            w_projb = const_pool.tile([128, 1], f32, tag="wpb")
            ones_col = const_pool.tile([128, 1], bf16, tag="ones")
            sdt = bf16 if _SST_BF16 else f32
            sstA = const_pool.tile([128, 64], sdt, tag="sstA")
            sstB = const_pool.tile([128, 64], sdt, tag="sstB")

            # Encoder weights first (the first steps need wih then whh);
            # decoder weights + projection load after the encoder loop is
            # issued, overlapping the 512 encoder steps.
            nc.sync.dma_start(w_enc_ih[:], d_enc_wih)
            # pk0 half first: the first rec matmuls only need cols 0..4095
            nc.sync.dma_start(w_enc_hh[:, 0:4096], d_enc_whh[:, 0:4096])
            nc.sync.dma_start(w_enc_hh[:, 4096:8192], d_enc_whh[:, 4096:8192])
            nc.vector.memset(ones_col[:], 1.0)
            nc.vector.memset(sstA[:], 0.0)
            nc.vector.memset(sstB[:], 0.0)
            # hT rings: per sample group, per k-half (k01 / k23) so the
            # pk0 recurrent matmuls only wait on the first half of hT.
            # Slot v holds hT of step u with u%DG == v, col k*16+j (2 k each).
            ringA0 = const_pool.tile([128, DG * 32], fp8, tag="ringA0")
            ringA1 = const_pool.tile([128, DG * 32], fp8, tag="ringA1")
            ringB0 = const_pool.tile([128, DG * 32], fp8, tag="ringB0")
            ringB1 = const_pool.tile([128, DG * 32], fp8, tag="ringB1")
            for r in (ringA0, ringA1, ringB0, ringB1):
                nc.vector.memset(r[:], 0.0)
            rings = ((ringA0, ringA1), (ringB0, ringB1))
            ssts = (sstA, sstB)

            def inproj(w_ih, psq, ohq, t, q):
                """Gate-PSUM init, one group: psq col = m*16 + j."""
                for m in range(16):
                    nc.tensor.matmul(
                        psq[:, m * 16:(m + 1) * 16],
                        w_ih[:, m * 128:(m + 1) * 128],
                        ohq[:, t * BS + q * 16: t * BS + q * 16 + 16],
                        start=True, stop=False, skip_group_check=True)

            def inproj2(w_ih, psf, ohq, t, q):
                """Gate-PSUM init for a 2-step flight (steps t, t+1), one
                group: psf [128, 512], col s*256 + m*16 + j."""
                ps3 = psf[:, :].rearrange("p (s mj) -> p s mj", s=2)
                oh4 = ohq[:, :].rearrange("p (f s q j) -> p f s q j",
                                          s=2, q=2, j=16)
                for m in range(16):
                    nc.tensor.matmul(
                        ps3[:, :, m * 16:(m + 1) * 16],
                        w_ih[:, m * 128:(m + 1) * 128],
                        oh4[:, t // 2, :, q, :],
                        start=True, stop=False, skip_group_check=True)

            def rec_mms(w_hh, psq, q, pv):
                """DoubleRow K=256 recurrent matmuls for one sample group.
                pk0 first across all m (it only needs the k01 ring half)."""
                for pk in range(2):
                    hprev = rings[q][pk][:, pv * 32:(pv + 1) * 32]
                    for m in range(16):
                        out = psq[:, m * 16:(m + 1) * 16]
                        w = w_hh[:, (pk * 16 + m) * 256:(pk * 16 + m + 1) * 256]
                        nc.tensor.matmul(
                            out,
                            w.rearrange("p (k c) -> p k c", k=2),
                            hprev.rearrange("p (k j) -> p k j", k=2),
                            start=False, stop=(pk == 1),
                            perf_mode=DR, skip_group_check=True)

            def chain(psqs, v):
                """Both groups' activation chains, stage-interleaved."""
                Ts, tc2s = [], []
                for q in range(2):
                    T = work_pool.tile([128, 256], bf16, tag=f"T{q}",
                                       name=f"T{q}")
                    nc.scalar.activation(T[:], psqs[q][:, :], AF.Tanh,
                                         scale=1.0 / WS)
                    Ts.append(T)
                a1eng = nc.gpsimd if _POOL_A1 else nc.vector
                a1s = []
                for q in range(2):
                    # a1 is off the c-critical path: compute it on Pool so
                    # the DVE a2->sst chain and Pool a1 run concurrently.
                    a1 = work_pool.tile([128, 64], f32, tag=f"a1{q}",
                                        name=f"a1{q}")
                    a1eng.scalar_tensor_tensor(a1[:], Ts[q][:, 64:128], 1.0,
                                               Ts[q][:, 0:64], OP.add,
                                               OP.mult)
                    a1s.append(a1)
                for q in range(2):
                    T, sst_q = Ts[q], ssts[q]
                    a2 = work_pool.tile([128, 64], f32, tag=f"a2{q}",
                                        name=f"a2{q}")
                    nc.vector.scalar_tensor_tensor(a2[:], T[:, 128:192], 1.0,
                                                   sst_q[:], OP.add, OP.mult)
                    nc.vector.scalar_tensor_tensor(sst_q[:], a2[:], 0.5,
                                                   a1s[q][:], OP.mult, OP.add)
                for q in range(2):
                    tc2 = work_pool.tile([128, 64], bf16, tag=f"tc2{q}",
                                         name=f"tc2{q}")
                    nc.scalar.activation(tc2[:], ssts[q][:], AF.Tanh, scale=0.5)
                    tc2s.append(tc2)
                for q in range(2):
                    for h in range(2):
                        nc.vector.scalar_tensor_tensor(
                            rings[q][h][:, v * 32:(v + 1) * 32],
                            Ts[q][:, 192 + h * 32:192 + (h + 1) * 32],
                            1.0, tc2s[q][:, h * 32:(h + 1) * 32],
                            OP.add, OP.mult)

            DGW = DG * 16  # per-group tail width

            def dec_tail_mm(g):
                """Batched logits matmuls for steps DG*g .. DG*g+DG-1."""
                ps_l = plog_pool.tile([128, 2 * DGW], f32, tag="psl")
                for q in range(2):
                    for k in range(4):
                        hsrc = rings[q][k // 2][:, :].rearrange(
                            "p (u k j) -> p u k j", u=DG, k=2, j=16)
                        nc.tensor.matmul(ps_l[:, q * DGW:(q + 1) * DGW],
                                         w_projt[:, k * 128:(k + 1) * 128],
                                         hsrc[:, :, k % 2, :],
                                         start=(k == 0), stop=(k == 3),
                                         skip_group_check=True)
                return ps_l

            def dec_tail_reduce(ps_l, ohq, gl, accum, g, i_sym):
                """exp / tgt-dot stack (bf16), ones-matmul, accum copy."""
                st = stack_pool.tile([128, 4 * DGW], bf16, tag="st")
                nc.scalar.activation(st[:, 0:2 * DGW], ps_l[:, :], AF.Exp,
                                     bias=w_projb[:, 0:1], scale=1.0)
                ohv = ohq[:, :].rearrange("p (t r) -> p t r", r=32)
                for q in range(2):
                    nc.vector.tensor_mul(
                        st[:, 2 * DGW + q * DGW:2 * DGW + (q + 1) * DGW]
                        .rearrange("p (v j) -> p v j", v=DG),
                        ps_l[:, q * DGW:(q + 1) * DGW]
                        .rearrange("p (v j) -> p v j", v=DG),
                        ohv[:, gl * DG:(gl + 1) * DG, q * 16:(q + 1) * 16])
                rp = prsp_pool.tile([1, 4 * DGW], f32, tag="rp")
                nc.tensor.matmul(rp[:, :], ones_col[:, 0:1], st[:, :],
                                 start=True, stop=True)
                nc.vector.tensor_copy(accum[:, g * 4 * DGW:(g + 1) * 4 * DGW],
                                      rp[:, :])

            def body(i, w_hh, w_ih, d_oh, dec):
                ohq = oh_pool.tile([128, U * BS], fp8, tag="oh")
                nc.sync.dma_start(ohq[:], d_oh[:, bass.ts(i, U * BS)])
                accum = None
                if dec:
                    accum = acc_pool.tile([1, U * 64], f32, tag="accum")
                if _FLIGHT_INPROJ:
                    psA = pgA_pool.tile([128, 512], f32, tag="psA")
                    psB = pgB_pool.tile([128, 512], f32, tag="psB")
                    inproj2(w_ih, psA, ohq, 0, 0)
                    inproj2(w_ih, psB, ohq, 0, 1)
                else:
                    psA = pgA_pool.tile([128, 256], f32, tag="psA")
                    psB = pgB_pool.tile([128, 256], f32, tag="psB")
                    inproj(w_ih, psA, ohq, 0, 0)
                    inproj(w_ih, psB, ohq, 0, 1)
                pend_mm = None
                pend_red = None
                for u in range(U):
                    g, v = u // DG, u % DG
                    pv = (u - 1) % DG
                    if _FLIGHT_INPROJ:
                        # Issue the new flight's input projections at the
                        # start of its first step: independent PE work sits
                        # in the FIFO exactly where the PE would otherwise
                        # stall waiting for the previous chain's ring write.
                        s = u % 2
                        if s == 0 and u > 0:
                            psA = pgA_pool.tile([128, 512], f32, tag="psA")
                            psB = pgB_pool.tile([128, 512], f32, tag="psB")
                            inproj2(w_ih, psA, ohq, u, 0)
                            inproj2(w_ih, psB, ohq, u, 1)
                        psqA = psA[:, s * 256:(s + 1) * 256]
                        psqB = psB[:, s * 256:(s + 1) * 256]
                    else:
                        psqA, psqB = psA, psB
                    psqs = (psqA, psqB)
                    rec_mms(w_hh, psqA, 0, pv)
                    rec_mms(w_hh, psqB, 1, pv)
                    # proj matmuls for the previous dec group run here: after
                    # this step's recurrent matmuls (so they don't delay them
                    # in the PE queue) but before this step's hT overwrites
                    # ring slot 0.
                    if pend_mm is not None and not _ABL_TAIL:
                        pend_red = (dec_tail_mm(pend_mm), ohq, pend_mm,
                                    accum, pend_mm, i)
                        pend_mm = None
                    pend_mm = None if _ABL_TAIL else pend_mm
                    if not _FLIGHT_INPROJ and u + 1 < U:
                        psA_n = pgA_pool.tile([128, 256], f32, tag="psA")
                        psB_n = pgB_pool.tile([128, 256], f32, tag="psB")
                        inproj(w_ih, psA_n, ohq, u + 1, 0)
                        inproj(w_ih, psB_n, ohq, u + 1, 1)
                    chain(psqs, v)
                    if pend_red is not None:
                        dec_tail_reduce(*pend_red)
                        pend_red = None
                    if dec and v == DG - 1:
                        pend_mm = g
                    if not _FLIGHT_INPROJ and u + 1 < U:
                        psA, psB = psA_n, psB_n
                if pend_mm is not None:
                    dec_tail_reduce(dec_tail_mm(pend_mm), ohq, pend_mm,
                                    accum, pend_mm, i)
                if dec:
                    nc.sync.dma_start(d_res[:, bass.ts(i, U * 64)],
                                      accum[:])

            _REPEAT = int(os.environ.get("REPEAT_MODULE", "1"))

            def phases():
                with tc.For_i(0, n_iters, 1, hint_engines=(PE,), name="enc") as i:
                    body(i, w_enc_hh, w_enc_ih, d_enc_oh, False)
                nc.sync.dma_start(w_dec_ih[:], d_dec_wih)
                nc.sync.dma_start(w_dec_hh[:], d_dec_whh)
                nc.sync.dma_start(w_projt[:], d_projt)
                nc.sync.dma_start(w_projb[:], d_projb)
                with tc.For_i(0, n_iters, 1, hint_engines=(PE,), name="dec") as i:
                    body(i, w_dec_hh, w_dec_ih, d_dec_oh, True)

            if unrolled:
                for i in range(n_iters):
                    body(i, w_enc_hh, w_enc_ih, d_enc_oh, False)
                nc.sync.dma_start(w_dec_ih[:], d_dec_wih)
                nc.sync.dma_start(w_dec_hh[:], d_dec_whh)
                nc.sync.dma_start(w_projt[:], d_projt)
                nc.sync.dma_start(w_projb[:], d_projb)
                for i in range(n_iters):
                    body(i, w_dec_hh, w_dec_ih, d_dec_oh, True)
            elif _REPEAT > 1:
                with tc.For_i(0, _REPEAT, 1, name="rep"):
                    phases()
            else:
                phases()

    nc.compile()
    return nc


def _prep_inmaps(inputs, n_steps=S):
    enc_wih, enc_whh = _prep_weights(inputs["enc_W_ih"], inputs["enc_W_hh"],
                                     inputs["enc_b_ih"], inputs["enc_b_hh"])
    dec_wih, dec_whh = _prep_weights(inputs["dec_W_ih"], inputs["dec_W_hh"],
                                     inputs["dec_b_ih"], inputs["dec_b_hh"])
    import ml_dtypes
    projW = 0.5 * np.asarray(inputs["proj_W"], np.float64)  # [128, 512]
    projt = (np.ascontiguousarray(projW.T).reshape(4, 128, 128)
             .transpose(1, 0, 2).reshape(128, 512).astype(ml_dtypes.bfloat16))
    projb = np.ascontiguousarray(
        np.asarray(inputs["proj_b"], np.float32).reshape(128, 1))

    C_idx = np.asarray(inputs["C_idx"])[:, :n_steps]
    Etgt = np.asarray(inputs["E"])[:, :n_steps]

    in_maps = []
    for c in range(NCORES):
        sl = slice(c * BS, (c + 1) * BS)
        in_maps.append({
            "enc_whh": enc_whh, "enc_wih": enc_wih,
            "dec_whh": dec_whh, "dec_wih": dec_wih,
            "projt": np.ascontiguousarray(projt), "projb": projb,
            "enc_oh": _onehot_stream(C_idx[sl]),
            "dec_oh": _onehot_stream(Etgt[sl]),
        })
    return in_maps


def _assemble(inputs, results, n_steps=S):
    """results: list of per-core {'res': ...} dicts."""
    Etgt = np.asarray(inputs["E"])[:, :n_steps]
    proj_b = np.asarray(inputs["proj_b"], np.float64)
    nll = np.empty((B, n_steps), np.float64)
    for c in range(NCORES):
        r = np.asarray(results[c]["res"], np.float64).reshape(
            n_steps // DG, 2, 2, DG, 16)       # [g, {sumexp,tgtdot}, q, v, j]
        r = r.transpose(0, 1, 3, 2, 4)         # [g, s, v, q, j]
        sumexp = r[:, 0].reshape(n_steps, BS)  # [u, sample 16q+j]
        tgtdot = r[:, 1].reshape(n_steps, BS)
        tgt = Etgt[c * BS:(c + 1) * BS]            # [j, u]
        bias_t = proj_b[tgt]                       # [j, u]
        nll[c * BS:(c + 1) * BS] = (np.log(sumexp).T
                                    - (tgtdot.T + bias_t))
    mask = (Etgt != 0).astype(np.float64)          # [B, u]
    num = (nll * mask).sum(axis=0)
    cnt = mask.sum(axis=0)
    step_loss = np.where(cnt > 0, num / np.maximum(cnt, 1.0), 0.0)
    return np.float32(step_loss.sum())


def _run(inputs, n_steps=S, trace=False):
    from concourse import bass_utils

    key = n_steps
    if key not in _CACHE:
        _CACHE[key] = _build_module(n_steps)
    nc = _CACHE[key]

    in_maps = _prep_inmaps(inputs, n_steps)
    res = bass_utils.run_bass_kernel_spmd(
        nc, in_maps, core_ids=list(range(NCORES)), trace=trace,
        trace_cores=[0] if trace else None)
    total = _assemble(inputs, res.results, n_steps)
    return total, res


def kernel(**inputs) -> np.ndarray:
    total, _ = _run(inputs, n_steps=S,
                    trace=bool(int(os.environ.get("LSTM_TRACE", "0"))))
    return np.float32(total)

